# revision 47
# baseline (speedup 1.0000x reference)
"""BinaryMatchAttention Trainium2 kernel.

reference semantics (per batch b):
    qb[k]   = (query_addr >> k) & 1                 k in [0, 16)
    w[s]    = prod_k (1 - |x[b, s, 96+k] - qb[k]|)
    out[b,d]= sum_s w[s] * x[b, s, d]               d in [0, 96)

Sharding: data-parallel over batch, one NeuronCore per batch element
(B == 8 == n_cores), no collectives.

Per-core plan (x_core [32768, 128] fp32 in HBM, memory-bound; the DMA
engines sustain ~420 GB/s with large descriptors):
  - flat row split: partition p holds the 256 consecutive seq rows
    s = 256p + i.  Each DMA wave loads an i-range for all partitions,
    so one descriptor per partition per wave moves rows*512 contiguous
    bytes (2-8 KiB descriptors; descriptor generation and per-packet
    overheads are negligible at this size).
  - waves are sized [16]*14 + [8,8,8,4,4] rows/partition: uniform
    8 KiB descriptors through the stream, ramped down at the end so the
    post-last-byte serial tail (weight chain + final matmuls + PSUM
    drain) is short.  Waves alternate between the two HWDGE rings
    (Sync / ACT).
  - match weights per wave on DVE: d = bits - qb, na = min(-d, d),
    t = 1 + na = 1 - |d|, then 4 strided pairwise products 16 -> 1.
  - einsum on TensorE: per 4-row group, psum[4, 384] += w4.T @ v[4, 96]
    (diagonal trick: only r==r' 96-blocks are wanted; host extracts).
    One PSUM accumulator across all 64 groups.
  - every cross-engine tile (vt, w) gets a unique buffer (no reuse), so
    no wave's DVE work ever head-of-line blocks on PE progress -- the
    only rate limiter is the HBM stream itself.
  - default build path is raw bass (no TileContext): manual 8-slot DMA
    completion sems + one cumulative DVE-progress sem, adjacent waves'
    DVE chains interleaved to hide sem-update latency, and a single
    all-engine barrier + sem range-clear at the end.  This avoids most
    of the TileContext build/teardown overhead (~13.4us exec for a
    trivial TileContext kernel).  BMA_IMPL=tile selects the TileContext
    path as a fallback.
  - float32r (TF32-like PE path) gives ~5e-4 rel err; "f32" exact mode
    is a fallback.
"""

import os
import sys

if "/opt/trn_rl_repo" not in sys.path:
    sys.path.insert(0, "/opt/trn_rl_repo")

import numpy as np

S, D = 32768, 128
VD = 96          # value payload dims
NBITS = 16
BIT0 = 96
P = 128          # partitions
R = 4            # rows per matmul group (diagonal trick)
C = R
IPP = S // P     # 256 rows per partition

# DMA wave sizes in rows-per-partition (one row = 512 B contiguous).
# Uniform 16-row waves keep the per-wave weight-chain latency low; the
# ramped tail shrinks the serial work left after the last byte lands.
_WAVE_PRESETS = {
    "fine": [16] * 14 + [8, 8, 8, 4, 4],
    "coarse": [16, 16, 32, 32, 32, 32, 32, 32, 16, 8, 4, 4],
    "mixed": [32] * 6 + [16] * 3 + [8, 8, 4, 4],
    "fine8": [8] * 30 + [4] * 4,
    "w24": [24] * 10 + [8, 4, 4],
    "rampin": [4, 4, 8] + [16] * 13 + [8, 8, 8, 4, 4],
}
WROWS = _WAVE_PRESETS[os.environ.get("BMA_WAVES", "fine")]
assert sum(WROWS) == IPP

NCORES = 8

# "f32r" : float32r matmuls (1 cycle/row, ~5e-4 rel err)
# "f32"  : plain fp32 matmuls (4 cycles/row, exact)
MM_MODE = os.environ.get("BMA_MM_MODE", "f32r")

_CACHE = {}


def _build(mode):
    import concourse.bacc as bacc
    import concourse.mybir as mybir
    import concourse.tile as tile

    f32 = mybir.dt.float32
    x_dt = mybir.dt.float32r if mode == "f32r" else f32

    nc = bacc.Bacc("TRN2", target_bir_lowering=False, debug=False)
    x = nc.dram_tensor("x", [S, D], x_dt, kind="ExternalInput")
    cq = nc.dram_tensor("cq", [P, NBITS], f32, kind="ExternalInput")
    out = nc.dram_tensor("out", [C, C * VD], f32, kind="ExternalOutput")

    # [128(part), 256(row), 128(col)]; rows of one partition are
    # contiguous in HBM, so each wave is one big descriptor per
    # partition.
    xr = x.ap().rearrange("(p i) d -> p i d", p=P)

    n_groups = IPP // R * P // P  # groups per partition-slice step
    last_g = (IPP // R) - 1

    with tile.TileContext(nc) as tc:
        with (
            tc.tile_pool(name="const", bufs=1) as cpool,
            tc.tile_pool(name="v32", bufs=max(1, WROWS.count(32))) as v32,
            tc.tile_pool(name="v16", bufs=max(1, WROWS.count(16))) as v16,
            tc.tile_pool(name="v8", bufs=max(1, WROWS.count(8))) as v8,
            tc.tile_pool(name="v4", bufs=max(1, WROWS.count(4))) as v4,
            tc.tile_pool(name="wk", bufs=3) as wk,
            tc.tile_pool(name="wp", bufs=len(WROWS)) as wp,
            tc.tile_pool(name="ps", bufs=1, space="PSUM") as ppool,
            tc.tile_pool(name="res", bufs=1) as rpool,
        ):
            cqt = cpool.tile([P, 1, NBITS], f32)
            nc.sync.dma_start(cqt[:], cq.ap().rearrange("p (a k) -> p a k", a=1))

            acc = ppool.tile([C, C * VD], f32)
            vpools = {32: v32, 16: v16, 8: v8, 4: v4}

            g = 0
            i0 = 0
            for ib, nr in enumerate(WROWS):
                vt = vpools[nr].tile([P, nr, D], x_dt, tag=f"vt{nr}")
                dma_eng = nc.sync if ib % 2 == 0 else nc.scalar
                dma_eng.dma_start(vt[:], xr[:, i0 : i0 + nr, :])
                i0 += nr

                bits = vt[:, :, BIT0 : BIT0 + NBITS]
                if mode == "f32r":
                    bits = bits.bitcast(f32)
                d = wk.tile([P, nr, NBITS], f32, tag="d")
                nc.vector.tensor_sub(d[:], bits, cqt[:].broadcast_to([P, nr, NBITS]))
                # na = min(-d, d) = -|d|; then t = 1 + na = 1 - |d|
                na = wk.tile([P, nr, NBITS], f32, tag="na")
                nc.vector.scalar_tensor_tensor(
                    na[:], d[:], -1.0, d[:],
                    op0=mybir.AluOpType.mult, op1=mybir.AluOpType.min,
                )
                t = wk.tile([P, nr, NBITS], f32, tag="t")
                nc.vector.tensor_scalar(
                    t[:], na[:], 1.0, None, op0=mybir.AluOpType.add,
                )
                p8 = wk.tile([P, nr, 8], f32, tag="p8")
                nc.vector.tensor_mul(p8[:], t[:, :, 0::2], t[:, :, 1::2])
                p4 = wk.tile([P, nr, 4], f32, tag="p4")
                nc.vector.tensor_mul(p4[:], p8[:, :, 0::2], p8[:, :, 1::2])
                p2 = wk.tile([P, nr, 2], f32, tag="p2")
                nc.vector.tensor_mul(p2[:], p4[:, :, 0::2], p4[:, :, 1::2])
                # final tree level rounds to the matmul dtype
                w = wp.tile([P, nr, 1], x_dt, tag="w")
                nc.vector.tensor_mul(w[:], p2[:, :, 0::2], p2[:, :, 1::2])

                for j in range(nr // R):
                    lhsT = w[:, j * R : (j + 1) * R, 0]      # [128, 4]
                    rhs = vt[:, j * R : (j + 1) * R, 0:VD]   # [128, 4, 96]
                    nc.tensor.matmul(
                        acc[:],
                        lhsT,
                        rhs,
                        start=(g == 0),
                        stop=(g == last_g),
                    )
                    g += 1

            res = rpool.tile([C, C * VD], f32)
            nc.vector.tensor_copy(res[:], acc[:])
            nc.sync.dma_start(out.ap(), res[:])

    nc.compile()
    return nc


def _build_raw(mode):
    """TileContext-free build: manual semaphores, no pool machinery, no
    build/build_end barrier blocks.  Saves ~8-10us of fixed framework
    overhead (measured ~13.4us exec for a trivial TileContext kernel)."""
    import concourse.bacc as bacc
    import concourse.mybir as mybir

    f32 = mybir.dt.float32
    x_dt = mybir.dt.float32r if mode == "f32r" else f32

    nc = bacc.Bacc("TRN2", target_bir_lowering=False, debug=False)
    x = nc.dram_tensor("x", [S, D], x_dt, kind="ExternalInput")
    cq = nc.dram_tensor("cq", [P, NBITS], f32, kind="ExternalInput")
    out = nc.dram_tensor("out", [C, C * VD], f32, kind="ExternalOutput")

    xr = x.ap().rearrange("(p i) d -> p i d", p=P)

    nw = len(WROWS)
    # 8 rotating DMA-completion sems (a single cumulative sem per ring is
    # unsound: per-engine FIFO allows mixed prefix sums to hit the target
    # with an incomplete middle wave).
    dsems = [nc.alloc_semaphore(f"dma{i}") for i in range(8)]
    duses = [0] * 8
    # One cumulative DVE-progress sem: engines are pipelined, so even
    # same-engine consumers must wait on the producer's sem update
    # (mirrors the tile framework's S[DVE]>=n chain).
    semDVE = nc.alloc_semaphore("dveprog")
    semGPS = nc.alloc_semaphore("gpsprog")
    semPE = nc.alloc_semaphore("pedone")
    # Odd tail waves' weight chains on GpSimd, concurrent with DVE's.
    # Pool rejects TensorScalarPtr ops, so the gps chain uses only
    # TensorTensor: 1-|b-q| = min((1+q)-b, b-(q-1)) with q+-1 tiles
    # precomputed once on DVE.
    gps_waves = sorted({13, 15, 17}) if os.environ.get("BMA_GPS") else []

    def dma(eng, dst, src, slot):
        if duses[slot]:
            # slot reuse: order the two uses so a later DMA's increments
            # can never satisfy an earlier DMA's wait target
            eng.wait_ge(dsems[slot], 16 * duses[slot])
        duses[slot] += 1
        eng.dma_start(dst, src).then_inc(dsems[slot], 16)
        return dsems[slot], 16 * duses[slot]

    cqt = nc.alloc_sbuf_tensor("cqt", [P, 1, NBITS], f32)
    vts = [
        nc.alloc_sbuf_tensor(f"vt{k}", [P, nr, D], x_dt)
        for k, nr in enumerate(WROWS)
    ]
    nrmax = max(WROWS)
    wk = {
        tag: [
            nc.alloc_sbuf_tensor(f"{tag}{i}", [P, nrmax, n], f32)
            for i in range(3)
        ]
        for tag, n in (
            ("d", NBITS), ("na", NBITS), ("t", NBITS),
            ("p8", 8), ("p4", 4), ("p2", 2),
        )
    }
    wts = [
        nc.alloc_sbuf_tensor(f"w{k}", [P, nr, 1], x_dt)
        for k, nr in enumerate(WROWS)
    ]
    res = nc.alloc_sbuf_tensor("res", [C, C * VD], f32)
    acc = nc.alloc_psum_tensor("acc", [C, C * VD], f32)

    # --- DMA issue streams (SP ring even waves + cq/out, ACT ring odd) ---
    cq_sem, cq_tgt = dma(
        nc.sync, cqt.ap(), cq.ap().rearrange("p (a k) -> p a k", a=1), 0
    )
    wave_done = []
    i0 = 0
    for k, nr in enumerate(WROWS):
        eng = nc.sync if k % 2 == 0 else nc.scalar
        wave_done.append(dma(eng, vts[k].ap(), xr[:, i0 : i0 + nr, :], (k + 1) % 8))
        i0 += nr

    # --- DVE: per-wave weight chain.  Every DVE op waits on its
    # predecessor's semDVE update (pipelined engine: program order alone
    # does not order SBUF reads after prior writes). ---
    dcnt = 0

    def dve(inst):
        nonlocal dcnt
        dcnt += 1
        inst.then_inc(semDVE, 1)
        return dcnt

    # Waves are processed in interleaved groups (pairs, and the last
    # three together): op N of wave b executes between op N and N+1 of
    # wave a, hiding the ~150ns sem-update propagation of each
    # producer->consumer hop behind the sibling wave's op.
    dve_list = [k for k in range(nw) if k not in gps_waves]
    groups = []
    k = 0
    while k < len(dve_list):
        take = 3 if len(dve_list) - k == 3 else (2 if len(dve_list) - k >= 2 else 1)
        groups.append(dve_list[k : k + take])
        k += take

    w_ready = [None] * nw

    def chain_ops(k):
        nr = WROWS[k]
        vt = vts[k]
        bits = vt.ap()[:, :, BIT0 : BIT0 + NBITS]
        if mode == "f32r":
            bits = bits.bitcast(f32)
        d = wk["d"][k % 3].ap()[:, 0:nr, :]
        na = wk["na"][k % 3].ap()[:, 0:nr, :]
        t = wk["t"][k % 3].ap()[:, 0:nr, :]
        p8 = wk["p8"][k % 3].ap()[:, 0:nr, :]
        p4 = wk["p4"][k % 3].ap()[:, 0:nr, :]
        p2 = wk["p2"][k % 3].ap()[:, 0:nr, :]
        w = wts[k].ap()
        yield lambda: dve(
            nc.vector.tensor_sub(d, bits, cqt.ap().broadcast_to([P, nr, NBITS]))
        )
        yield lambda: dve(nc.vector.scalar_tensor_tensor(
            na, d, -1.0, d, op0=mybir.AluOpType.mult, op1=mybir.AluOpType.min
        ))
        yield lambda: dve(
            nc.vector.tensor_scalar(t, na, 1.0, None, op0=mybir.AluOpType.add)
        )
        yield lambda: dve(nc.vector.tensor_mul(p8, t[:, :, 0::2], t[:, :, 1::2]))
        yield lambda: dve(nc.vector.tensor_mul(p4, p8[:, :, 0::2], p8[:, :, 1::2]))
        yield lambda: dve(nc.vector.tensor_mul(p2, p4[:, :, 0::2], p4[:, :, 1::2]))
        yield lambda: dve(nc.vector.tensor_mul(w, p2[:, :, 0::2], p2[:, :, 1::2]))

    if gps_waves:
        # q+1 / q-1 const tiles, computed once on DVE (dve counts 1, 2)
        q1p = nc.alloc_sbuf_tensor("q1p", [P, 1, NBITS], f32)
        qm1 = nc.alloc_sbuf_tensor("qm1", [P, 1, NBITS], f32)
        nc.vector.wait_ge(cq_sem, cq_tgt)
        dve(nc.vector.tensor_scalar(
            q1p.ap(), cqt.ap(), 1.0, None, op0=mybir.AluOpType.add))
        dve(nc.vector.tensor_scalar(
            qm1.ap(), cqt.ap(), -1.0, None, op0=mybir.AluOpType.add))
        gmax = max(WROWS[k] for k in gps_waves)
        wkg = {
            tag: [
                nc.alloc_sbuf_tensor(f"g{tag}{i}", [P, gmax, n], f32)
                for i in range(len(gps_waves))
            ]
            for tag, n in (("u", NBITS), ("v", NBITS), ("m", NBITS),
                           ("p8", 8), ("p4", 4), ("p2", 2))
        }

    prev_cnt = {}
    for grp in groups:
        chains = {}
        for k in grp:
            sem, tgt = wave_done[k]
            nc.vector.wait_ge(sem, tgt)
            if k == 0:
                nc.vector.wait_ge(cq_sem, cq_tgt)
            chains[k] = chain_ops(k)
            prev_cnt[k] = None
        for step in range(7):
            for k in grp:
                if prev_cnt[k] is not None:
                    nc.vector.wait_ge(semDVE, prev_cnt[k])
                prev_cnt[k] = next(chains[k])()
                if step == 6:
                    w_ready[k] = (semDVE, prev_cnt[k])

    # --- GpSimd chains for the gps waves (3-way interleaved) ---
    gcnt = 0
    if gps_waves:
        eng = nc.gpsimd
        eng.wait_ge(semDVE, 2)  # q1p/qm1 ready
        gchains = {}
        gprev = {}

        def gps_ops(k, gi):
            nr = WROWS[k]
            vt = vts[k]
            bits = vt.ap()[:, :, BIT0 : BIT0 + NBITS]
            if mode == "f32r":
                bits = bits.bitcast(f32)
            u = wkg["u"][gi].ap()[:, 0:nr, :]
            v = wkg["v"][gi].ap()[:, 0:nr, :]
            m = wkg["m"][gi].ap()[:, 0:nr, :]
            p8 = wkg["p8"][gi].ap()[:, 0:nr, :]
            p4 = wkg["p4"][gi].ap()[:, 0:nr, :]
            p2 = wkg["p2"][gi].ap()[:, 0:nr, :]
            w = wts[k].ap()
            q1b = q1p.ap().broadcast_to([P, nr, NBITS])
            qmb = qm1.ap().broadcast_to([P, nr, NBITS])
            yield lambda: eng.tensor_sub(u, q1b, bits)
            yield lambda: eng.tensor_sub(v, bits, qmb)
            yield lambda: eng.tensor_tensor(m, u, v, op=mybir.AluOpType.min)
            yield lambda: eng.tensor_mul(p8, m[:, :, 0::2], m[:, :, 1::2])
            yield lambda: eng.tensor_mul(p4, p8[:, :, 0::2], p8[:, :, 1::2])
            yield lambda: eng.tensor_mul(p2, p4[:, :, 0::2], p4[:, :, 1::2])
            yield lambda: eng.tensor_mul(w, p2[:, :, 0::2], p2[:, :, 1::2])

        for gi, k in enumerate(gps_waves):
            gchains[k] = gps_ops(k, gi)
            gprev[k] = None
        for step in range(7):
            for k in gps_waves:
                if step == 0:
                    sem, tgt = wave_done[k]
                    eng.wait_ge(sem, tgt)
                if gprev[k] is not None:
                    eng.wait_ge(semGPS, gprev[k])
                inst = next(gchains[k])()
                gcnt += 1
                inst.then_inc(semGPS, 1)
                gprev[k] = gcnt
                if step == 6:
                    w_ready[k] = (semGPS, gcnt)

    # --- PE: ordered PSUM accumulation, one wait per wave ---
    g = 0
    last_g = (IPP // R) - 1
    for k, nr in enumerate(WROWS):
        nc.tensor.wait_ge(*w_ready[k])
        for j in range(nr // R):
            mm = nc.tensor.matmul(
                acc.ap(),
                wts[k].ap()[:, j * R : (j + 1) * R, 0],
                vts[k].ap()[:, j * R : (j + 1) * R, 0:VD],
                start=(g == 0),
                stop=(g == last_g),
            )
            g += 1
    mm.then_inc(semPE, 1)

    # --- drain: PSUM -> SBUF -> HBM ---
    nc.vector.wait_ge(semPE, 1)
    res_done = dve(nc.vector.tensor_copy(res.ap(), acc.ap()))
    nc.sync.wait_ge(semDVE, res_done)
    out_sem, out_tgt = dma(nc.sync, out.ap(), res.ap(), 1)
    # restore sems to 0 so NEFF re-execution starts clean: SP observes
    # every final value (incl. the out receipt), then range-clears.
    # BMA_NOCLEAR=1 skips all of it, relying on the NEFF exit protocol's
    # own DMA-queue quiesce + fresh sem state per execution.
    if not os.environ.get("BMA_NOCLEAR"):
        nc.sync.wait_ge(out_sem, out_tgt)
        for i, s in enumerate(dsems):
            if duses[i]:
                nc.sync.wait_ge(s, 16 * duses[i])
        nc.sync.wait_ge(semDVE, dcnt)
        if gcnt:
            nc.sync.wait_ge(semGPS, gcnt)
        nc.sync.wait_ge(semPE, 1)
        # No all-engine barrier: SP has observed every sem's final value,
        # so no engine can update them again; the other engines' streams
        # have retired (their last instructions produced those values).
        # The race detector is conservative here; BMA_BARRIER=1 restores it.
        if os.environ.get("BMA_BARRIER"):
            nc.all_engine_barrier()
        all_sems = dsems + [semDVE, semGPS, semPE]
        lo = min(s.num for s in all_sems)
        hi = max(s.num for s in all_sems)
        nc.sync.drain(semaphore_range=range(lo, hi + 1))
        nc.sync.sem_clear(range(lo, hi + 1))

    nc.compile()
    return nc


def _get_nc(mode):
    impl = os.environ.get("BMA_IMPL", "raw")
    key = (mode, impl)
    if key not in _CACHE:
        _CACHE[key] = _build_raw(mode) if impl == "raw" else _build(mode)
    return _CACHE[key]


def run(x, query_addr, trace=False, mode=None):
    """Returns (output [B, 96] float32, BassKernelResults)."""
    from concourse.bass_utils import run_bass_kernel_spmd

    mode = mode or MM_MODE
    x = np.asarray(x)
    qa = int(np.asarray(query_addr))
    assert x.shape == (NCORES, S, D), x.shape

    qb = np.array([(qa >> k) & 1 for k in range(NBITS)], dtype=np.float32)
    cq = np.ascontiguousarray(np.broadcast_to(qb, (P, NBITS)))

    nc = _get_nc(mode)
    in_maps = [
        {"x": np.ascontiguousarray(x[b], dtype=np.float32), "cq": cq}
        for b in range(NCORES)
    ]
    if not trace:
        # A stray BASS_TRACE in the env would route run_bass_kernel_spmd
        # into the NTFF-hook path, which needs antenv.axon_hooks (absent
        # in this image unless test.py installs a shim).
        os.environ["BASS_NEVER_TRACE"] = "1"
    else:
        os.environ.pop("BASS_NEVER_TRACE", None)
    kres = run_bass_kernel_spmd(nc, in_maps, list(range(NCORES)), trace=trace)

    outs = []
    for r in kres.results:
        o = np.asarray(r["out"]).reshape(C, C, VD)
        outs.append(o[np.arange(C), np.arange(C)].sum(axis=0))
    return np.stack(outs).astype(np.float32), kres


def kernel(x, query_addr):
    return run(x, query_addr)[0]
